# revision 25
# baseline (speedup 1.0000x reference)
"""CoLightGAT forward on 8 Trainium2 NeuronCores (Bass/Tile).

Strategy (pure data parallelism, batch sharded 8 ways):
  - Only node 0's GAT output row is needed, so attention reduces to a
    5-way softmax per (batch, head) and a weighted sum of encoder
    embeddings (gat_w folded into the value/policy head weights on host).
  - On-chip pipeline per core (Bc = 2048 batch):
      enc1 (PE, fp32r)  -> relu (ACT, -> bf16)
      W2aug (PE, bf16): emb_noBias row-major + attention logit dots
      softmax over 5 neighbors (DVE/ACT, batch on partitions)
      block-diag attn matrices (GPSIMD affine_select)
      ctx^T = emb_chunk^T @ diag  (PE, fp32r, PSUM-accumulated over nodes)
      value/policy hidden (PE fp32r) -> relu (ACT, -> bf16)
      output matmuls row-major [128b, 9] (PE, bf16) -> +bias -> DMA out
"""

import numpy as np
import ml_dtypes

N_CORES = 8
B_TOTAL = 16384
OBS = 64
ACTD = 8
H = 128
NN = 4
NNODE = 5
HEADS = 2
SLOPE = 0.2

BC = B_TOTAL // N_CORES      # 2048 batch per core
P = 128                      # partitions / batch tile
T = BC // P                  # 16 tiles per core
NCHUNK = T * NNODE           # 80 node-row chunks of 128
EW = H + 2 * HEADS           # 132: emb cols + [u1_0,u2_0,u1_1,u2_1]
BIG = 60.0

_CACHE = {}


def _mkap(base_ap, col_off, pairs):
    """AP over an SBUF tile: keep partition dim, custom free dims."""
    import concourse.bass as bass
    return bass.AP(
        tensor=base_ap.tensor,
        offset=base_ap.offset + col_off,
        ap=[list(base_ap.ap[0])] + [list(p) for p in pairs],
    )


def _build(bc=BC):
    import concourse.bacc as bacc
    import concourse.tile as tile
    import concourse.mybir as mybir
    import concourse.bass as bass

    dt = mybir.dt
    AF = mybir.ActivationFunctionType
    ALU = mybir.AluOpType
    f32, f32r, bf16, i32 = dt.float32, dt.float32r, dt.bfloat16, dt.int32

    t_ = bc // P                 # tiles
    nchunk = t_ * NNODE          # chunks of 128 node-rows
    ncol = nchunk * P            # node-row columns total
    ngrp = (nchunk + 2) // 3     # emb psum groups of 3 chunks

    nc = bacc.Bacc("TRN2", target_bir_lowering=False, debug=False,
                   num_devices=N_CORES)

    # ---- DRAM tensors ----
    # aobsT: host-transposed [obs | neighbors] in feature-major, node-major
    # column order: col (t*5 + c)*128 + b  <->  batch row t*128+b, node c.
    d_aobsT = nc.dram_tensor("aobsT", [OBS, ncol], bf16,
                             kind="ExternalInput").ap()
    d_adj = nc.dram_tensor("adj", [bc, NNODE], i32, kind="ExternalInput").ap()
    d_w1 = nc.dram_tensor("w1", [OBS, H], bf16, kind="ExternalInput").ap()
    d_b1 = nc.dram_tensor("b1", [H, 1], f32, kind="ExternalInput").ap()
    d_w2aug = nc.dram_tensor("w2aug", [H, EW], bf16, kind="ExternalInput").ap()
    d_kb = nc.dram_tensor("kb", [1, HEADS], f32, kind="ExternalInput").ap()
    d_bv = nc.dram_tensor("bv", [H, HEADS * H], bf16, kind="ExternalInput").ap()
    d_bp = nc.dram_tensor("bp", [H, HEADS * H], bf16, kind="ExternalInput").ap()
    d_vb1 = nc.dram_tensor("vb1", [H, 1], f32, kind="ExternalInput").ap()
    d_pb1 = nc.dram_tensor("pb1", [H, 1], f32, kind="ExternalInput").ap()
    d_wv2 = nc.dram_tensor("wv2", [H, 1], bf16, kind="ExternalInput").ap()
    d_wp2 = nc.dram_tensor("wp2", [H, ACTD], bf16, kind="ExternalInput").ap()
    d_outb = nc.dram_tensor("outb", [1, 1 + ACTD], f32, kind="ExternalInput").ap()
    d_ident = nc.dram_tensor("ident", [P, P], bf16, kind="ExternalInput").ap()
    d_val = nc.dram_tensor("value", [bc, 1], f32, kind="ExternalOutput").ap()
    d_log = nc.dram_tensor("logits", [bc, ACTD], f32, kind="ExternalOutput").ap()

    QT = 4                       # tiles per pipeline quarter
    nq = (t_ + QT - 1) // QT     # quarters

    with tile.TileContext(nc) as tc:
        with (
            tc.tile_pool(name="persist", bufs=1) as pp,
            tc.tile_pool(name="diagp", bufs=4) as dp,
            tc.tile_pool(name="scratch", bufs=3) as sp,
            nc.psum_tensor([P, 2048], f32) as psA,
            nc.psum_tensor([P, 2048], f32) as psB,
        ):
            # PSUM bank map: A0,A1 enc1 | A2,A3 ctx | B0-B2 emb | B3 heads
            # outs reuse A0/A1 (late in the timeline).
            # ---- persistent SBUF ----
            aobsT = pp.tile([OBS, ncol], bf16)
            relu1 = pp.tile([P, ncol], bf16)
            emb = pp.tile([P, nchunk * EW], bf16)
            attn = pp.tile([P, t_ * 2 * NNODE], f32)   # col = t*10 + h*5 + c
            ctx = pp.tile([P, HEADS * bc], bf16)       # col = h*bc + t*128 + b
            vh = pp.tile([P, bc], bf16)
            ph = pp.tile([P, bc], bf16)
            outs = pp.tile([P, t_ * 9], f32)
            adjm1 = pp.tile([P, t_ * NNODE], f32)

            w1_sb = pp.tile([OBS, H], bf16)
            b1_sb = pp.tile([H, 1], f32)
            w2aug_sb = pp.tile([H, EW], bf16)
            kb_sb = pp.tile([P, HEADS], f32)
            bv_sb = pp.tile([H, HEADS * H], bf16)
            bp_sb = pp.tile([H, HEADS * H], bf16)
            vb1_sb = pp.tile([H, 1], f32)
            pb1_sb = pp.tile([H, 1], f32)
            wv2_sb = pp.tile([H, 1], bf16)
            wp2_sb = pp.tile([H, ACTD], bf16)
            outb_sb = pp.tile([P, 1 + ACTD], f32)
            ident_sb = pp.tile([P, P], bf16)
            adj_sb = pp.tile([P, t_ * NNODE], i32)
            zcol = pp.tile([P, 1], f32)
            nc.vector.memset(zcol[:], 0.0)

            qcol = QT * NNODE * P        # aobsT cols per quarter

            # ---- DMA: enc1-critical weights first, then activations ----
            q0c = min(qcol, ncol)
            nc.sync.dma_start(out=w1_sb[:], in_=d_w1[:])
            nc.sync.dma_start(out=b1_sb[:], in_=d_b1[:])
            nc.sync.dma_start(out=aobsT[:, 0:q0c // 2],
                              in_=d_aobsT[:, 0:q0c // 2])
            nc.sync.dma_start(out=aobsT[:, q0c // 2:q0c],
                              in_=d_aobsT[:, q0c // 2:q0c])
            # remaining weights on the ACT HWDGE queue (parallel issue)
            for dst, src in ((w2aug_sb, d_w2aug), (bv_sb, d_bv),
                             (bp_sb, d_bp), (vb1_sb, d_vb1),
                             (pb1_sb, d_pb1), (wv2_sb, d_wv2),
                             (wp2_sb, d_wp2), (ident_sb, d_ident)):
                nc.scalar.dma_start(out=dst[:], in_=src[:])
            nc.scalar.dma_start(out=kb_sb[:], in_=d_kb.partition_broadcast(P))
            nc.scalar.dma_start(out=outb_sb[:], in_=d_outb.partition_broadcast(P))
            adj_src = bass.AP(
                tensor=d_adj.tensor, offset=0,
                ap=[[NNODE, P], [NNODE * P, t_], [1, NNODE]],
            )
            nc.scalar.dma_start(out=adj_sb[:], in_=adj_src)
            for q in range(1, nq):
                hi = min((q + 1) * qcol, ncol)
                nc.sync.dma_start(out=aobsT[:, q * qcol:hi],
                                  in_=d_aobsT[:, q * qcol:hi])
            nc.vector.tensor_scalar_add(adjm1[:], adj_sb[:], -1.0)

            ei = 0     # enc1 psum rotation
            gi = 0     # emb-group psum rotation

            def ph_front(q):
                nonlocal ei, gi
                tlo, thi = q * QT, min((q + 1) * QT, t_)
                clo, chi = tlo * NNODE * P, thi * NNODE * P
                klo, khi = tlo * NNODE, thi * NNODE
                # encoder layer 1 + relu (split evac ACT/DVE)
                for c0 in range(clo, chi, 512):
                    w = min(512, chi - c0)
                    pv = psA[:, (ei % 2) * 512:(ei % 2) * 512 + w]
                    ei += 1
                    nc.tensor.matmul(pv, w1_sb[:], aobsT[:, c0:c0 + w],
                                     start=True, stop=True)
                    hw_ = w // 2
                    nc.scalar.activation(out=relu1[:, c0:c0 + hw_],
                                         in_=pv[:, 0:hw_], func=AF.Relu,
                                         bias=b1_sb[:], scale=1.0)
                    nc.vector.scalar_tensor_tensor(
                        out=relu1[:, c0 + hw_:c0 + w], in0=pv[:, hw_:w],
                        scalar=b1_sb[:, 0:1],
                        in1=_mkap(zcol[:], 0, [[0, w - hw_]]),
                        op0=ALU.add, op1=ALU.max)
                # emb_noBias + logit dots (W2aug), groups of 3
                for k0 in range(klo, khi, 3):
                    ks = range(k0, min(k0 + 3, khi))
                    gw = len(ks) * EW
                    gp = psB[:, (gi % 3) * 512:(gi % 3) * 512 + gw]
                    for idx, k in enumerate(ks):
                        nc.tensor.matmul(
                            gp[:, idx * EW:(idx + 1) * EW],
                            relu1[:, k * P:(k + 1) * P],
                            w2aug_sb[:],
                            start=True, stop=True)
                    if gi % 2 == 0:
                        nc.scalar.activation(
                            out=emb[:, k0 * EW:k0 * EW + gw],
                            in_=gp, func=AF.Copy, bias=0.0, scale=1.0)
                    else:
                        nc.vector.tensor_copy(
                            emb[:, k0 * EW:k0 * EW + gw], gp)
                    gi += 1

            def ph_attn(q):
                tlo, thi = q * QT, min((q + 1) * QT, t_)
                klo, khi = tlo * NNODE, thi * NNODE
                qt = thi - tlo
                qn = qt * NNODE
                # softmax over neighbors (no max-sub; |e| <= ~10)
                e_s = sp.tile([P, qn], f32, tag="e_s")
                ex_s = sp.tile([P, qn], f32, tag="ex_s")
                for h in range(HEADS):
                    ejap = _mkap(emb[:], klo * EW + 129 + 2 * h,
                                 [[EW * NNODE, qt], [EW, NNODE]])
                    ei0 = _mkap(emb[:], klo * EW + 128 + 2 * h,
                                [[EW * NNODE, qt], [0, NNODE]])
                    nc.vector.scalar_tensor_tensor(
                        out=e_s[:], in0=ejap, scalar=kb_sb[:, h:h + 1],
                        in1=ei0, op0=ALU.add, op1=ALU.add)
                    nc.vector.scalar_tensor_tensor(
                        out=e_s[:], in0=e_s[:], scalar=SLOPE, in1=e_s[:],
                        op0=ALU.mult, op1=ALU.max)
                    nc.vector.scalar_tensor_tensor(
                        out=e_s[:], in0=adjm1[:, klo:khi], scalar=BIG,
                        in1=e_s[:], op0=ALU.mult, op1=ALU.add)
                    nc.scalar.activation(out=ex_s[:], in_=e_s[:], func=AF.Exp)
                    sm = sp.tile([P, qt], tag="sm", dtype=f32)
                    nc.vector.tensor_reduce(
                        out=sm[:],
                        in_=ex_s[:].rearrange("p (t c) -> p t c", c=NNODE),
                        axis=mybir.AxisListType.X, op=ALU.add)
                    rc = sp.tile([P, qt], tag="rc", dtype=f32)
                    nc.vector.reciprocal(rc[:], sm[:])
                    rcb = _mkap(rc[:], 0, [[1, qt], [0, NNODE]])
                    attn_out = _mkap(attn[:], tlo * 2 * NNODE + h * NNODE,
                                     [[2 * NNODE, qt], [1, NNODE]])
                    nc.vector.tensor_tensor(attn_out, ex_s[:], rcb,
                                            op=ALU.mult)
                # attention apply per tile
                for t in range(tlo, thi):
                    diag = dp.tile([P, NNODE * HEADS * P], bf16, tag="diag")
                    attn_in = _mkap(attn[:], t * 2 * NNODE, [[1, NNODE], [0, P]])
                    diag_out = _mkap(diag[:], 0, [[HEADS * P, NNODE], [1, P]])
                    nc.gpsimd.affine_select(
                        out=diag_out, in_=attn_in,
                        pattern=[[0, NNODE], [1, P]],
                        compare_op=ALU.is_equal, fill=0.0,
                        base=0, channel_multiplier=-1)
                    attn_in1 = _mkap(attn[:], t * 2 * NNODE + NNODE,
                                     [[1, NNODE], [0, P]])
                    diag_out1 = _mkap(diag[:], P, [[HEADS * P, NNODE], [1, P]])
                    nc.gpsimd.affine_select(
                        out=diag_out1, in_=attn_in1,
                        pattern=[[0, NNODE], [1, P]],
                        compare_op=ALU.is_equal, fill=0.0,
                        base=0, channel_multiplier=-1)
                    pair, half = (t // 2) % 2, t % 2
                    cbase = 1024 + pair * 512 + half * HEADS * P
                    cps = psA[:, cbase:cbase + HEADS * P]
                    for c in range(NNODE):
                        k = t * NNODE + c
                        nc.tensor.matmul(
                            cps,
                            emb[:, k * EW:k * EW + H],
                            diag[:, c * HEADS * P:(c + 1) * HEADS * P],
                            start=(c == 0), stop=(c == NNODE - 1))
                    if half == 1:
                        t0 = t - 1
                        ctx_out = _mkap(ctx[:], t0 * P,
                                        [[P, 2], [bc, HEADS], [1, P]])
                        ps_in = bass.AP(
                            tensor=psA.ap().tensor,
                            offset=1024 + pair * 512,
                            ap=[list(psA.ap().ap[0]),
                                [HEADS * P, 2], [P, HEADS], [1, P]])
                        nc.vector.tensor_copy(ctx_out, ps_in)

            def ph_tail(q):
                tlo, thi = q * QT, min((q + 1) * QT, t_)
                qt = thi - tlo
                # value / policy hidden (bank B3)
                i0, i1 = tlo * P, thi * P
                for (w_sb, bias_sb, out_bf) in ((bv_sb, vb1_sb, vh),
                                                (bp_sb, pb1_sb, ph)):
                    pv = psB[:, 1536:1536 + (i1 - i0)]
                    for h in range(HEADS):
                        nc.tensor.matmul(
                            pv,
                            w_sb[:, h * H:(h + 1) * H],
                            ctx[:, h * bc + i0:h * bc + i1],
                            start=(h == 0), stop=(h == HEADS - 1))
                    nc.scalar.activation(out=out_bf[:, i0:i1], in_=pv,
                                         func=AF.Relu, bias=bias_sb[:],
                                         scale=1.0)
                # output layer [128b, 9] (banks A0/A1)
                ob = (q % 2) * 512
                for s in range(tlo, thi):
                    o0 = ob + (s - tlo) * 9
                    nc.tensor.matmul(psA[:, o0:o0 + 1],
                                     vh[:, s * P:(s + 1) * P],
                                     wv2_sb[:], start=True, stop=True)
                    nc.tensor.matmul(psA[:, o0 + 1:o0 + 9],
                                     ph[:, s * P:(s + 1) * P],
                                     wp2_sb[:], start=True, stop=True)
                outb_b = _mkap(outb_sb[:], 0, [[0, qt], [1, 9]])
                ps_in = bass.AP(tensor=psA.ap().tensor, offset=ob,
                                ap=[list(psA.ap().ap[0]), [9, qt], [1, 9]])
                nc.vector.tensor_tensor(
                    _mkap(outs[:], tlo * 9, [[9, qt], [1, 9]]),
                    ps_in, outb_b, op=ALU.add)
                # outputs to DRAM
                val_src = _mkap(outs[:], tlo * 9, [[9, qt]])
                val_dst = bass.AP(tensor=d_val.tensor, offset=tlo * P,
                                  ap=[[1, P], [P, qt]])
                nc.sync.dma_start(out=val_dst, in_=val_src)
                log_src = _mkap(outs[:], tlo * 9 + 1, [[9, qt], [1, ACTD]])
                log_dst = bass.AP(tensor=d_log.tensor,
                                  offset=tlo * P * ACTD,
                                  ap=[[ACTD, P], [ACTD * P, qt], [1, ACTD]])
                nc.sync.dma_start(out=log_dst, in_=log_src)

            # software-pipelined emission: skew phases by one/two quarters
            for w in range(nq + 2):
                if w < nq:
                    ph_front(w)
                if 1 <= w <= nq:
                    ph_attn(w - 1)
                if w >= 2:
                    ph_tail(w - 2)

    nc.compile()
    nc._dbg_aps = {
        "aobsT": aobsT, "relu1": relu1, "emb": emb, "attn": attn,
        "ctx": ctx, "vh": vh, "ph": ph, "outs": outs, "adjm1": adjm1,
    }
    return nc


def _precompute(enc_w1, enc_b1, enc_w2, enc_b2, gat_w, gat_a,
                val_w1, val_b1, val_w2, val_b2,
                pol_w1, pol_b1, pol_w2, pol_b2):
    bfdt = ml_dtypes.bfloat16
    f = np.float32
    a1, a2 = gat_a[:, :H].astype(f), gat_a[:, H:].astype(f)
    va1 = np.stack([gat_w[h] @ a1[h] for h in range(HEADS)])
    va2 = np.stack([gat_w[h] @ a2[h] for h in range(HEADS)])
    u1 = np.stack([enc_w2 @ va1[h] for h in range(HEADS)])
    u2 = np.stack([enc_w2 @ va2[h] for h in range(HEADS)])
    kb = np.array([[enc_b2 @ va1[h] + enc_b2 @ va2[h]
                    for h in range(HEADS)]], f)
    w2aug = np.concatenate(
        [enc_w2] + [c[:, None] for pair in zip(u1, u2) for c in pair],
        axis=1).astype(f)
    bv = np.concatenate([gat_w[h] @ val_w1[h * H:(h + 1) * H]
                         for h in range(HEADS)], axis=1).astype(f)
    bp = np.concatenate([gat_w[h] @ pol_w1[h * H:(h + 1) * H]
                         for h in range(HEADS)], axis=1).astype(f)
    vb1 = (val_b1 + enc_b2 @ (bv[:, :H] + bv[:, H:])).astype(f)[:, None]
    pb1 = (pol_b1 + enc_b2 @ (bp[:, :H] + bp[:, H:])).astype(f)[:, None]
    outb = np.concatenate([val_b2, pol_b2]).astype(f)[None, :]
    return {
        "w1": enc_w1.astype(bfdt), "b1": enc_b1.astype(f)[:, None],
        "w2aug": w2aug.astype(bfdt), "kb": kb,
        "bv": bv.astype(bfdt), "bp": bp.astype(bfdt), "vb1": vb1, "pb1": pb1,
        "wv2": val_w2.astype(bfdt), "wp2": pol_w2.astype(bfdt),
        "outb": outb, "ident": np.eye(P, dtype=np.float32).astype(bfdt),
    }


def kernel(obs, neighbor_obs, adj,
           enc_w1, enc_b1, enc_w2, enc_b2, gat_w, gat_a,
           val_w1, val_b1, val_w2, val_b2,
           pol_w1, pol_b1, pol_w2, pol_b2,
           _trace=False, _trace_kwargs=None):
    from concourse.bass_utils import run_bass_kernel_spmd

    if "nc" not in _CACHE:
        _CACHE["nc"] = _build()
    nc = _CACHE["nc"]

    wmap = _precompute(enc_w1, enc_b1, enc_w2, enc_b2, gat_w, gat_a,
                       val_w1, val_b1, val_w2, val_b2,
                       pol_w1, pol_b1, pol_w2, pol_b2)
    obs = np.asarray(obs, np.float32)
    nbr = np.asarray(neighbor_obs, np.float32)
    adj = np.ascontiguousarray(np.asarray(adj, np.int32))

    # [B,5,64] -> per-core [64, T*5*128], node-major columns
    aobs = np.concatenate([obs[:, None, :], nbr], axis=1)
    aobsT = np.ascontiguousarray(
        aobs.reshape(N_CORES, BC // P, P, NNODE, OBS)
            .transpose(0, 4, 1, 3, 2)
            .reshape(N_CORES, OBS, BC * NNODE)
            .astype(ml_dtypes.bfloat16))

    in_maps = []
    for c in range(N_CORES):
        s = slice(c * BC, (c + 1) * BC)
        in_maps.append({"aobsT": aobsT[c], "adj": adj[s], **wmap})

    kw = {}
    if _trace:
        kw = dict(trace=True, **(_trace_kwargs or {}))
    res = run_bass_kernel_spmd(nc, in_maps, list(range(N_CORES)), **kw)
    value = np.concatenate([r["value"] for r in res.results], axis=0)
    logits = np.concatenate([r["logits"] for r in res.results], axis=0)
    _CACHE["last_results"] = res
    return value, logits


# revision 26
# speedup vs baseline: 1.0513x; 1.0513x over previous
"""CoLightGAT forward on 8 Trainium2 NeuronCores (Bass/Tile).

Strategy (pure data parallelism, batch sharded 8 ways):
  - Only node 0's GAT output row is needed, so attention reduces to a
    5-way softmax per (batch, head) and a weighted sum of encoder
    embeddings (gat_w folded into the value/policy head weights on host).
  - On-chip pipeline per core (Bc = 2048 batch):
      enc1 (PE, fp32r)  -> relu (ACT, -> bf16)
      W2aug (PE, bf16): emb_noBias row-major + attention logit dots
      softmax over 5 neighbors (DVE/ACT, batch on partitions)
      block-diag attn matrices (GPSIMD affine_select)
      ctx^T = emb_chunk^T @ diag  (PE, fp32r, PSUM-accumulated over nodes)
      value/policy hidden (PE fp32r) -> relu (ACT, -> bf16)
      output matmuls row-major [128b, 9] (PE, bf16) -> +bias -> DMA out
"""

import numpy as np
import ml_dtypes

N_CORES = 8
B_TOTAL = 16384
OBS = 64
ACTD = 8
H = 128
NN = 4
NNODE = 5
HEADS = 2
SLOPE = 0.2

BC = B_TOTAL // N_CORES      # 2048 batch per core
P = 128                      # partitions / batch tile
T = BC // P                  # 16 tiles per core
NCHUNK = T * NNODE           # 80 node-row chunks of 128
EW = H + 2 * HEADS           # 132: emb cols + [u1_0,u2_0,u1_1,u2_1]
BIG = 60.0

_CACHE = {}


def _mkap(base_ap, col_off, pairs):
    """AP over an SBUF tile: keep partition dim, custom free dims."""
    import concourse.bass as bass
    return bass.AP(
        tensor=base_ap.tensor,
        offset=base_ap.offset + col_off,
        ap=[list(base_ap.ap[0])] + [list(p) for p in pairs],
    )


def _build(bc=BC):
    import concourse.bacc as bacc
    import concourse.tile as tile
    import concourse.mybir as mybir
    import concourse.bass as bass

    dt = mybir.dt
    AF = mybir.ActivationFunctionType
    ALU = mybir.AluOpType
    f32, f32r, bf16, i32 = dt.float32, dt.float32r, dt.bfloat16, dt.int32

    t_ = bc // P                 # tiles
    nchunk = t_ * NNODE          # chunks of 128 node-rows
    ncol = nchunk * P            # node-row columns total
    ngrp = (nchunk + 2) // 3     # emb psum groups of 3 chunks

    nc = bacc.Bacc("TRN2", target_bir_lowering=False, debug=False,
                   num_devices=N_CORES)

    # ---- DRAM tensors ----
    # aobsT: host-transposed [obs | neighbors] in feature-major, node-major
    # column order: col (t*5 + c)*128 + b  <->  batch row t*128+b, node c.
    d_aobsT = nc.dram_tensor("aobsT", [OBS, ncol], bf16,
                             kind="ExternalInput").ap()
    d_adj = nc.dram_tensor("adj", [bc, NNODE], i32, kind="ExternalInput").ap()
    d_w1 = nc.dram_tensor("w1", [OBS, H], bf16, kind="ExternalInput").ap()
    d_b1 = nc.dram_tensor("b1", [H, 1], f32, kind="ExternalInput").ap()
    d_w2aug = nc.dram_tensor("w2aug", [H, EW], bf16, kind="ExternalInput").ap()
    d_kb = nc.dram_tensor("kb", [1, HEADS], f32, kind="ExternalInput").ap()
    d_bv = nc.dram_tensor("bv", [H, HEADS * H], bf16, kind="ExternalInput").ap()
    d_bp = nc.dram_tensor("bp", [H, HEADS * H], bf16, kind="ExternalInput").ap()
    d_vb1 = nc.dram_tensor("vb1", [H, 1], f32, kind="ExternalInput").ap()
    d_pb1 = nc.dram_tensor("pb1", [H, 1], f32, kind="ExternalInput").ap()
    d_wv2 = nc.dram_tensor("wv2", [H, 1], bf16, kind="ExternalInput").ap()
    d_wp2 = nc.dram_tensor("wp2", [H, ACTD], bf16, kind="ExternalInput").ap()
    d_outb = nc.dram_tensor("outb", [1, 1 + ACTD], f32, kind="ExternalInput").ap()
    d_ident = nc.dram_tensor("ident", [P, P], bf16, kind="ExternalInput").ap()
    d_val = nc.dram_tensor("value", [bc, 1], f32, kind="ExternalOutput").ap()
    d_log = nc.dram_tensor("logits", [bc, ACTD], f32, kind="ExternalOutput").ap()

    QT = 4                       # tiles per pipeline quarter
    nq = (t_ + QT - 1) // QT     # quarters

    with tile.TileContext(nc) as tc:
        with (
            tc.tile_pool(name="persist", bufs=1) as pp,
            tc.tile_pool(name="diagp", bufs=4) as dp,
            tc.tile_pool(name="scratch", bufs=3) as sp,
            nc.psum_tensor([P, 2048], f32) as psA,
            nc.psum_tensor([P, 2048], f32) as psB,
        ):
            # PSUM bank map: A0,A1 enc1 | A2,A3 ctx | B0-B2 emb | B3 heads
            # outs reuse A0/A1 (late in the timeline).
            # ---- persistent SBUF ----
            aobsT = pp.tile([OBS, ncol], bf16)
            relu1 = pp.tile([P, ncol], bf16)
            emb = pp.tile([P, nchunk * EW], bf16)
            attn = pp.tile([P, t_ * 2 * NNODE], f32)   # col = t*10 + h*5 + c
            ctx = pp.tile([P, HEADS * bc], bf16)       # col = h*bc + t*128 + b
            vh = pp.tile([P, bc], bf16)
            ph = pp.tile([P, bc], bf16)
            outs = pp.tile([P, t_ * 9], f32)
            adjm1 = pp.tile([P, t_ * NNODE], f32)

            w1_sb = pp.tile([OBS, H], bf16)
            b1_sb = pp.tile([H, 1], f32)
            w2aug_sb = pp.tile([H, EW], bf16)
            kb_sb = pp.tile([P, HEADS], f32)
            bv_sb = pp.tile([H, HEADS * H], bf16)
            bp_sb = pp.tile([H, HEADS * H], bf16)
            vb1_sb = pp.tile([H, 1], f32)
            pb1_sb = pp.tile([H, 1], f32)
            wv2_sb = pp.tile([H, 1], bf16)
            wp2_sb = pp.tile([H, ACTD], bf16)
            outb_sb = pp.tile([P, 1 + ACTD], f32)
            ident_sb = pp.tile([P, P], bf16)
            adj_sb = pp.tile([P, t_ * NNODE], i32)
            zcol = pp.tile([P, 1], f32)
            nc.vector.memset(zcol[:], 0.0)

            qcol = QT * NNODE * P        # aobsT cols per quarter

            # ---- DMA: first quarter's activations, then weights ----
            q0c = min(qcol, ncol)
            nc.sync.dma_start(out=aobsT[:, 0:q0c // 2],
                              in_=d_aobsT[:, 0:q0c // 2])
            nc.sync.dma_start(out=aobsT[:, q0c // 2:q0c],
                              in_=d_aobsT[:, q0c // 2:q0c])
            for dst, src in ((w1_sb, d_w1), (b1_sb, d_b1),
                             (w2aug_sb, d_w2aug), (bv_sb, d_bv),
                             (bp_sb, d_bp), (vb1_sb, d_vb1),
                             (pb1_sb, d_pb1), (wv2_sb, d_wv2),
                             (wp2_sb, d_wp2), (ident_sb, d_ident)):
                nc.sync.dma_start(out=dst[:], in_=src[:])
            nc.sync.dma_start(out=kb_sb[:], in_=d_kb.partition_broadcast(P))
            nc.sync.dma_start(out=outb_sb[:], in_=d_outb.partition_broadcast(P))
            for q in range(1, nq):
                hi = min((q + 1) * qcol, ncol)
                nc.sync.dma_start(out=aobsT[:, q * qcol:hi],
                                  in_=d_aobsT[:, q * qcol:hi])
            # adj [bc,5] -> [128, (t,5)]  (vector-engine HWDGE queue)
            adj_src = bass.AP(
                tensor=d_adj.tensor, offset=0,
                ap=[[NNODE, P], [NNODE * P, t_], [1, NNODE]],
            )
            nc.scalar.dma_start(out=adj_sb[:], in_=adj_src)
            nc.vector.tensor_scalar_add(adjm1[:], adj_sb[:], -1.0)

            ei = 0     # enc1 psum rotation
            gi = 0     # emb-group psum rotation

            def ph_front(q):
                nonlocal ei, gi
                tlo, thi = q * QT, min((q + 1) * QT, t_)
                clo, chi = tlo * NNODE * P, thi * NNODE * P
                klo, khi = tlo * NNODE, thi * NNODE
                # encoder layer 1 + relu (split evac ACT/DVE)
                for c0 in range(clo, chi, 512):
                    w = min(512, chi - c0)
                    pv = psA[:, (ei % 2) * 512:(ei % 2) * 512 + w]
                    ei += 1
                    nc.tensor.matmul(pv, w1_sb[:], aobsT[:, c0:c0 + w],
                                     start=True, stop=True)
                    hw_ = w // 2
                    nc.scalar.activation(out=relu1[:, c0:c0 + hw_],
                                         in_=pv[:, 0:hw_], func=AF.Relu,
                                         bias=b1_sb[:], scale=1.0)
                    nc.vector.scalar_tensor_tensor(
                        out=relu1[:, c0 + hw_:c0 + w], in0=pv[:, hw_:w],
                        scalar=b1_sb[:, 0:1],
                        in1=_mkap(zcol[:], 0, [[0, w - hw_]]),
                        op0=ALU.add, op1=ALU.max)
                # emb_noBias + logit dots (W2aug), groups of 3
                for k0 in range(klo, khi, 3):
                    ks = range(k0, min(k0 + 3, khi))
                    gw = len(ks) * EW
                    gp = psB[:, (gi % 3) * 512:(gi % 3) * 512 + gw]
                    for idx, k in enumerate(ks):
                        nc.tensor.matmul(
                            gp[:, idx * EW:(idx + 1) * EW],
                            relu1[:, k * P:(k + 1) * P],
                            w2aug_sb[:],
                            start=True, stop=True)
                    if gi % 2 == 0:
                        nc.scalar.activation(
                            out=emb[:, k0 * EW:k0 * EW + gw],
                            in_=gp, func=AF.Copy, bias=0.0, scale=1.0)
                    else:
                        nc.vector.tensor_copy(
                            emb[:, k0 * EW:k0 * EW + gw], gp)
                    gi += 1

            def ph_attn(q):
                tlo, thi = q * QT, min((q + 1) * QT, t_)
                klo, khi = tlo * NNODE, thi * NNODE
                qt = thi - tlo
                qn = qt * NNODE
                # softmax over neighbors (no max-sub; |e| <= ~10)
                e_s = sp.tile([P, qn], f32, tag="e_s")
                ex_s = sp.tile([P, qn], f32, tag="ex_s")
                for h in range(HEADS):
                    ejap = _mkap(emb[:], klo * EW + 129 + 2 * h,
                                 [[EW * NNODE, qt], [EW, NNODE]])
                    ei0 = _mkap(emb[:], klo * EW + 128 + 2 * h,
                                [[EW * NNODE, qt], [0, NNODE]])
                    nc.vector.scalar_tensor_tensor(
                        out=e_s[:], in0=ejap, scalar=kb_sb[:, h:h + 1],
                        in1=ei0, op0=ALU.add, op1=ALU.add)
                    nc.vector.scalar_tensor_tensor(
                        out=e_s[:], in0=e_s[:], scalar=SLOPE, in1=e_s[:],
                        op0=ALU.mult, op1=ALU.max)
                    nc.vector.scalar_tensor_tensor(
                        out=e_s[:], in0=adjm1[:, klo:khi], scalar=BIG,
                        in1=e_s[:], op0=ALU.mult, op1=ALU.add)
                    nc.scalar.activation(out=ex_s[:], in_=e_s[:], func=AF.Exp)
                    sm = sp.tile([P, qt], tag="sm", dtype=f32)
                    nc.vector.tensor_reduce(
                        out=sm[:],
                        in_=ex_s[:].rearrange("p (t c) -> p t c", c=NNODE),
                        axis=mybir.AxisListType.X, op=ALU.add)
                    rc = sp.tile([P, qt], tag="rc", dtype=f32)
                    nc.vector.reciprocal(rc[:], sm[:])
                    rcb = _mkap(rc[:], 0, [[1, qt], [0, NNODE]])
                    attn_out = _mkap(attn[:], tlo * 2 * NNODE + h * NNODE,
                                     [[2 * NNODE, qt], [1, NNODE]])
                    nc.vector.tensor_tensor(attn_out, ex_s[:], rcb,
                                            op=ALU.mult)
                # attention apply per tile
                for t in range(tlo, thi):
                    diag = dp.tile([P, NNODE * HEADS * P], bf16, tag="diag")
                    attn_in = _mkap(attn[:], t * 2 * NNODE, [[1, NNODE], [0, P]])
                    diag_out = _mkap(diag[:], 0, [[HEADS * P, NNODE], [1, P]])
                    nc.gpsimd.affine_select(
                        out=diag_out, in_=attn_in,
                        pattern=[[0, NNODE], [1, P]],
                        compare_op=ALU.is_equal, fill=0.0,
                        base=0, channel_multiplier=-1)
                    attn_in1 = _mkap(attn[:], t * 2 * NNODE + NNODE,
                                     [[1, NNODE], [0, P]])
                    diag_out1 = _mkap(diag[:], P, [[HEADS * P, NNODE], [1, P]])
                    nc.gpsimd.affine_select(
                        out=diag_out1, in_=attn_in1,
                        pattern=[[0, NNODE], [1, P]],
                        compare_op=ALU.is_equal, fill=0.0,
                        base=0, channel_multiplier=-1)
                    pair, half = (t // 2) % 2, t % 2
                    cbase = 1024 + pair * 512 + half * HEADS * P
                    cps = psA[:, cbase:cbase + HEADS * P]
                    for c in range(NNODE):
                        k = t * NNODE + c
                        nc.tensor.matmul(
                            cps,
                            emb[:, k * EW:k * EW + H],
                            diag[:, c * HEADS * P:(c + 1) * HEADS * P],
                            start=(c == 0), stop=(c == NNODE - 1))
                    if half == 1:
                        t0 = t - 1
                        ctx_out = _mkap(ctx[:], t0 * P,
                                        [[P, 2], [bc, HEADS], [1, P]])
                        ps_in = bass.AP(
                            tensor=psA.ap().tensor,
                            offset=1024 + pair * 512,
                            ap=[list(psA.ap().ap[0]),
                                [HEADS * P, 2], [P, HEADS], [1, P]])
                        nc.vector.tensor_copy(ctx_out, ps_in)

            def ph_tail(q):
                tlo, thi = q * QT, min((q + 1) * QT, t_)
                qt = thi - tlo
                # value / policy hidden (bank B3)
                i0, i1 = tlo * P, thi * P
                for (w_sb, bias_sb, out_bf) in ((bv_sb, vb1_sb, vh),
                                                (bp_sb, pb1_sb, ph)):
                    pv = psB[:, 1536:1536 + (i1 - i0)]
                    for h in range(HEADS):
                        nc.tensor.matmul(
                            pv,
                            w_sb[:, h * H:(h + 1) * H],
                            ctx[:, h * bc + i0:h * bc + i1],
                            start=(h == 0), stop=(h == HEADS - 1))
                    nc.scalar.activation(out=out_bf[:, i0:i1], in_=pv,
                                         func=AF.Relu, bias=bias_sb[:],
                                         scale=1.0)
                # output layer [128b, 9] (banks A0/A1)
                ob = (q % 2) * 512
                for s in range(tlo, thi):
                    o0 = ob + (s - tlo) * 9
                    nc.tensor.matmul(psA[:, o0:o0 + 1],
                                     vh[:, s * P:(s + 1) * P],
                                     wv2_sb[:], start=True, stop=True)
                    nc.tensor.matmul(psA[:, o0 + 1:o0 + 9],
                                     ph[:, s * P:(s + 1) * P],
                                     wp2_sb[:], start=True, stop=True)
                outb_b = _mkap(outb_sb[:], 0, [[0, qt], [1, 9]])
                ps_in = bass.AP(tensor=psA.ap().tensor, offset=ob,
                                ap=[list(psA.ap().ap[0]), [9, qt], [1, 9]])
                nc.vector.tensor_tensor(
                    _mkap(outs[:], tlo * 9, [[9, qt], [1, 9]]),
                    ps_in, outb_b, op=ALU.add)
                # outputs to DRAM
                val_src = _mkap(outs[:], tlo * 9, [[9, qt]])
                val_dst = bass.AP(tensor=d_val.tensor, offset=tlo * P,
                                  ap=[[1, P], [P, qt]])
                nc.sync.dma_start(out=val_dst, in_=val_src)
                log_src = _mkap(outs[:], tlo * 9 + 1, [[9, qt], [1, ACTD]])
                log_dst = bass.AP(tensor=d_log.tensor,
                                  offset=tlo * P * ACTD,
                                  ap=[[ACTD, P], [ACTD * P, qt], [1, ACTD]])
                nc.sync.dma_start(out=log_dst, in_=log_src)

            # software-pipelined emission: skew phases by one/two quarters
            for w in range(nq + 2):
                if w < nq:
                    ph_front(w)
                if 1 <= w <= nq:
                    ph_attn(w - 1)
                if w >= 2:
                    ph_tail(w - 2)

    nc.compile()
    nc._dbg_aps = {
        "aobsT": aobsT, "relu1": relu1, "emb": emb, "attn": attn,
        "ctx": ctx, "vh": vh, "ph": ph, "outs": outs, "adjm1": adjm1,
    }
    return nc


def _precompute(enc_w1, enc_b1, enc_w2, enc_b2, gat_w, gat_a,
                val_w1, val_b1, val_w2, val_b2,
                pol_w1, pol_b1, pol_w2, pol_b2):
    bfdt = ml_dtypes.bfloat16
    f = np.float32
    a1, a2 = gat_a[:, :H].astype(f), gat_a[:, H:].astype(f)
    va1 = np.stack([gat_w[h] @ a1[h] for h in range(HEADS)])
    va2 = np.stack([gat_w[h] @ a2[h] for h in range(HEADS)])
    u1 = np.stack([enc_w2 @ va1[h] for h in range(HEADS)])
    u2 = np.stack([enc_w2 @ va2[h] for h in range(HEADS)])
    kb = np.array([[enc_b2 @ va1[h] + enc_b2 @ va2[h]
                    for h in range(HEADS)]], f)
    w2aug = np.concatenate(
        [enc_w2] + [c[:, None] for pair in zip(u1, u2) for c in pair],
        axis=1).astype(f)
    bv = np.concatenate([gat_w[h] @ val_w1[h * H:(h + 1) * H]
                         for h in range(HEADS)], axis=1).astype(f)
    bp = np.concatenate([gat_w[h] @ pol_w1[h * H:(h + 1) * H]
                         for h in range(HEADS)], axis=1).astype(f)
    vb1 = (val_b1 + enc_b2 @ (bv[:, :H] + bv[:, H:])).astype(f)[:, None]
    pb1 = (pol_b1 + enc_b2 @ (bp[:, :H] + bp[:, H:])).astype(f)[:, None]
    outb = np.concatenate([val_b2, pol_b2]).astype(f)[None, :]
    return {
        "w1": enc_w1.astype(bfdt), "b1": enc_b1.astype(f)[:, None],
        "w2aug": w2aug.astype(bfdt), "kb": kb,
        "bv": bv.astype(bfdt), "bp": bp.astype(bfdt), "vb1": vb1, "pb1": pb1,
        "wv2": val_w2.astype(bfdt), "wp2": pol_w2.astype(bfdt),
        "outb": outb, "ident": np.eye(P, dtype=np.float32).astype(bfdt),
    }


def kernel(obs, neighbor_obs, adj,
           enc_w1, enc_b1, enc_w2, enc_b2, gat_w, gat_a,
           val_w1, val_b1, val_w2, val_b2,
           pol_w1, pol_b1, pol_w2, pol_b2,
           _trace=False, _trace_kwargs=None):
    from concourse.bass_utils import run_bass_kernel_spmd

    if "nc" not in _CACHE:
        _CACHE["nc"] = _build()
    nc = _CACHE["nc"]

    wmap = _precompute(enc_w1, enc_b1, enc_w2, enc_b2, gat_w, gat_a,
                       val_w1, val_b1, val_w2, val_b2,
                       pol_w1, pol_b1, pol_w2, pol_b2)
    obs = np.asarray(obs, np.float32)
    nbr = np.asarray(neighbor_obs, np.float32)
    adj = np.ascontiguousarray(np.asarray(adj, np.int32))

    # [B,5,64] -> per-core [64, T*5*128], node-major columns
    aobs = np.concatenate([obs[:, None, :], nbr], axis=1)
    aobsT = np.ascontiguousarray(
        aobs.reshape(N_CORES, BC // P, P, NNODE, OBS)
            .transpose(0, 4, 1, 3, 2)
            .reshape(N_CORES, OBS, BC * NNODE)
            .astype(ml_dtypes.bfloat16))

    in_maps = []
    for c in range(N_CORES):
        s = slice(c * BC, (c + 1) * BC)
        in_maps.append({"aobsT": aobsT[c], "adj": adj[s], **wmap})

    kw = {}
    if _trace:
        kw = dict(trace=True, **(_trace_kwargs or {}))
    res = run_bass_kernel_spmd(nc, in_maps, list(range(N_CORES)), **kw)
    value = np.concatenate([r["value"] for r in res.results], axis=0)
    logits = np.concatenate([r["logits"] for r in res.results], axis=0)
    _CACHE["last_results"] = res
    return value, logits


# revision 27
# speedup vs baseline: 1.0637x; 1.0119x over previous
"""CoLightGAT forward on 8 Trainium2 NeuronCores (Bass/Tile).

Strategy (pure data parallelism, batch sharded 8 ways):
  - Only node 0's GAT output row is needed, so attention reduces to a
    5-way softmax per (batch, head) and a weighted sum of encoder
    embeddings (gat_w folded into the value/policy head weights on host).
  - On-chip pipeline per core (Bc = 2048 batch):
      enc1 (PE, fp32r)  -> relu (ACT, -> bf16)
      W2aug (PE, bf16): emb_noBias row-major + attention logit dots
      softmax over 5 neighbors (DVE/ACT, batch on partitions)
      block-diag attn matrices (GPSIMD affine_select)
      ctx^T = emb_chunk^T @ diag  (PE, fp32r, PSUM-accumulated over nodes)
      value/policy hidden (PE fp32r) -> relu (ACT, -> bf16)
      output matmuls row-major [128b, 9] (PE, bf16) -> +bias -> DMA out
"""

import numpy as np
import ml_dtypes

N_CORES = 8
B_TOTAL = 16384
OBS = 64
ACTD = 8
H = 128
NN = 4
NNODE = 5
HEADS = 2
SLOPE = 0.2

BC = B_TOTAL // N_CORES      # 2048 batch per core
P = 128                      # partitions / batch tile
T = BC // P                  # 16 tiles per core
NCHUNK = T * NNODE           # 80 node-row chunks of 128
EW = H + 2 * HEADS           # 132: emb cols + [u1_0,u2_0,u1_1,u2_1]
BIG = 60.0

_CACHE = {}


def _mkap(base_ap, col_off, pairs):
    """AP over an SBUF tile: keep partition dim, custom free dims."""
    import concourse.bass as bass
    return bass.AP(
        tensor=base_ap.tensor,
        offset=base_ap.offset + col_off,
        ap=[list(base_ap.ap[0])] + [list(p) for p in pairs],
    )


def _build(bc=BC):
    import concourse.bacc as bacc
    import concourse.tile as tile
    import concourse.mybir as mybir
    import concourse.bass as bass

    dt = mybir.dt
    AF = mybir.ActivationFunctionType
    ALU = mybir.AluOpType
    f32, f32r, bf16, i32 = dt.float32, dt.float32r, dt.bfloat16, dt.int32

    t_ = bc // P                 # tiles
    nchunk = t_ * NNODE          # chunks of 128 node-rows
    ncol = nchunk * P            # node-row columns total
    ngrp = (nchunk + 2) // 3     # emb psum groups of 3 chunks

    nc = bacc.Bacc("TRN2", target_bir_lowering=False, debug=False,
                   num_devices=N_CORES)

    # ---- DRAM tensors ----
    # aobsT: host-transposed [obs | neighbors] in feature-major, node-major
    # column order: col (t*5 + c)*128 + b  <->  batch row t*128+b, node c.
    d_aobsT = nc.dram_tensor("aobsT", [OBS, ncol], bf16,
                             kind="ExternalInput").ap()
    d_adj = nc.dram_tensor("adj", [bc, NNODE], i32, kind="ExternalInput").ap()
    # all weights packed host-side into two tensors (2 DMAs, not 14)
    WPB = H + EW + 2 * HEADS * H + 1 + ACTD + P      # 909 bf16 cols
    WPF = 3 + HEADS + 1 + ACTD                       # 14 f32 cols
    d_wpb = nc.dram_tensor("wpb", [P, WPB], bf16, kind="ExternalInput").ap()
    d_wpf = nc.dram_tensor("wpf", [P, WPF], f32, kind="ExternalInput").ap()
    d_val = nc.dram_tensor("value", [bc, 1], f32, kind="ExternalOutput").ap()
    d_log = nc.dram_tensor("logits", [bc, ACTD], f32, kind="ExternalOutput").ap()

    QT = 4                       # tiles per pipeline quarter
    nq = (t_ + QT - 1) // QT     # quarters

    with tile.TileContext(nc) as tc:
        with (
            tc.tile_pool(name="persist", bufs=1) as pp,
            tc.tile_pool(name="diagp", bufs=4) as dp,
            tc.tile_pool(name="scratch", bufs=3) as sp,
            nc.psum_tensor([P, 2048], f32) as psA,
            nc.psum_tensor([P, 2048], f32) as psB,
        ):
            # PSUM bank map: A0,A1 enc1 | A2,A3 ctx | B0-B2 emb | B3 heads
            # outs reuse A0/A1 (late in the timeline).
            # ---- persistent SBUF ----
            aobsT = pp.tile([OBS, ncol], bf16)
            relu1 = pp.tile([P, ncol], bf16)
            emb = pp.tile([P, nchunk * EW], bf16)
            attn = pp.tile([P, t_ * 2 * NNODE], f32)   # col = t*10 + h*5 + c
            ctx = pp.tile([P, HEADS * bc], bf16)       # col = h*bc + t*128 + b
            vh = pp.tile([P, bc], bf16)
            ph = pp.tile([P, bc], bf16)
            outs = pp.tile([P, t_ * 9], f32)
            adjm1 = pp.tile([P, t_ * NNODE], f32)

            wpb = pp.tile([P, WPB], bf16)
            wpf = pp.tile([P, WPF], f32)
            o = 0
            w1_sb = wpb[0:OBS, o:o + H]; o += H
            w2aug_sb = wpb[:, o:o + EW]; o += EW
            bv_sb = wpb[:, o:o + HEADS * H]; o += HEADS * H
            bp_sb = wpb[:, o:o + HEADS * H]; o += HEADS * H
            wv2_sb = wpb[:, o:o + 1]; o += 1
            wp2_sb = wpb[:, o:o + ACTD]; o += ACTD
            ident_sb = wpb[:, o:o + P]; o += P
            b1_sb = wpf[:, 0:1]
            vb1_sb = wpf[:, 1:2]
            pb1_sb = wpf[:, 2:3]
            kb_sb = wpf[:, 3:3 + HEADS]
            outb_sb = wpf[:, 3 + HEADS:3 + HEADS + 1 + ACTD]
            adj_sb = pp.tile([P, t_ * NNODE], i32)
            zcol = pp.tile([P, 1], f32)
            nc.vector.memset(zcol[:], 0.0)

            qcol = QT * NNODE * P        # aobsT cols per quarter

            # ---- DMA: weight packs + first-quarter activations first ----
            q0c = min(qcol, ncol)
            nc.sync.dma_start(out=wpb[:], in_=d_wpb[:])
            nc.sync.dma_start(out=wpf[:], in_=d_wpf[:])
            nc.sync.dma_start(out=aobsT[:, 0:q0c // 2],
                              in_=d_aobsT[:, 0:q0c // 2])
            nc.sync.dma_start(out=aobsT[:, q0c // 2:q0c],
                              in_=d_aobsT[:, q0c // 2:q0c])
            for q in range(1, nq):
                hi = min((q + 1) * qcol, ncol)
                nc.sync.dma_start(out=aobsT[:, q * qcol:hi],
                                  in_=d_aobsT[:, q * qcol:hi])
            adj_src = bass.AP(
                tensor=d_adj.tensor, offset=0,
                ap=[[NNODE, P], [NNODE * P, t_], [1, NNODE]],
            )
            nc.scalar.dma_start(out=adj_sb[:], in_=adj_src)
            nc.vector.tensor_scalar_add(adjm1[:], adj_sb[:], -1.0)

            ei = 0     # enc1 psum rotation
            gi = 0     # emb-group psum rotation

            def ph_front(q):
                nonlocal ei, gi
                tlo, thi = q * QT, min((q + 1) * QT, t_)
                clo, chi = tlo * NNODE * P, thi * NNODE * P
                klo, khi = tlo * NNODE, thi * NNODE
                # encoder layer 1 + relu (split evac ACT/DVE)
                for c0 in range(clo, chi, 512):
                    w = min(512, chi - c0)
                    pv = psA[:, (ei % 2) * 512:(ei % 2) * 512 + w]
                    ei += 1
                    nc.tensor.matmul(pv, w1_sb[:], aobsT[:, c0:c0 + w],
                                     start=True, stop=True)
                    hw_ = w // 2
                    nc.scalar.activation(out=relu1[:, c0:c0 + hw_],
                                         in_=pv[:, 0:hw_], func=AF.Relu,
                                         bias=b1_sb[:], scale=1.0)
                    nc.vector.scalar_tensor_tensor(
                        out=relu1[:, c0 + hw_:c0 + w], in0=pv[:, hw_:w],
                        scalar=b1_sb[:, 0:1],
                        in1=_mkap(zcol[:], 0, [[0, w - hw_]]),
                        op0=ALU.add, op1=ALU.max)
                # emb_noBias + logit dots (W2aug), groups of 3
                for k0 in range(klo, khi, 3):
                    ks = range(k0, min(k0 + 3, khi))
                    gw = len(ks) * EW
                    gp = psB[:, (gi % 3) * 512:(gi % 3) * 512 + gw]
                    for idx, k in enumerate(ks):
                        nc.tensor.matmul(
                            gp[:, idx * EW:(idx + 1) * EW],
                            relu1[:, k * P:(k + 1) * P],
                            w2aug_sb[:],
                            start=True, stop=True)
                    if gi % 2 == 0:
                        nc.scalar.activation(
                            out=emb[:, k0 * EW:k0 * EW + gw],
                            in_=gp, func=AF.Copy, bias=0.0, scale=1.0)
                    else:
                        nc.vector.tensor_copy(
                            emb[:, k0 * EW:k0 * EW + gw], gp)
                    gi += 1

            def ph_attn(q):
                tlo, thi = q * QT, min((q + 1) * QT, t_)
                klo, khi = tlo * NNODE, thi * NNODE
                qt = thi - tlo
                qn = qt * NNODE
                # softmax over neighbors (no max-sub; |e| <= ~10)
                e_s = sp.tile([P, qn], f32, tag="e_s")
                ex_s = sp.tile([P, qn], f32, tag="ex_s")
                for h in range(HEADS):
                    ejap = _mkap(emb[:], klo * EW + 129 + 2 * h,
                                 [[EW * NNODE, qt], [EW, NNODE]])
                    ei0 = _mkap(emb[:], klo * EW + 128 + 2 * h,
                                [[EW * NNODE, qt], [0, NNODE]])
                    nc.vector.scalar_tensor_tensor(
                        out=e_s[:], in0=ejap, scalar=kb_sb[:, h:h + 1],
                        in1=ei0, op0=ALU.add, op1=ALU.add)
                    nc.vector.scalar_tensor_tensor(
                        out=e_s[:], in0=e_s[:], scalar=SLOPE, in1=e_s[:],
                        op0=ALU.mult, op1=ALU.max)
                    nc.vector.scalar_tensor_tensor(
                        out=e_s[:], in0=adjm1[:, klo:khi], scalar=BIG,
                        in1=e_s[:], op0=ALU.mult, op1=ALU.add)
                    nc.scalar.activation(out=ex_s[:], in_=e_s[:], func=AF.Exp)
                    sm = sp.tile([P, qt], tag="sm", dtype=f32)
                    nc.vector.tensor_reduce(
                        out=sm[:],
                        in_=ex_s[:].rearrange("p (t c) -> p t c", c=NNODE),
                        axis=mybir.AxisListType.X, op=ALU.add)
                    rc = sp.tile([P, qt], tag="rc", dtype=f32)
                    nc.vector.reciprocal(rc[:], sm[:])
                    rcb = _mkap(rc[:], 0, [[1, qt], [0, NNODE]])
                    attn_out = _mkap(attn[:], tlo * 2 * NNODE + h * NNODE,
                                     [[2 * NNODE, qt], [1, NNODE]])
                    nc.vector.tensor_tensor(attn_out, ex_s[:], rcb,
                                            op=ALU.mult)
                # attention apply per tile
                for t in range(tlo, thi):
                    diag = dp.tile([P, NNODE * HEADS * P], bf16, tag="diag")
                    attn_in = _mkap(attn[:], t * 2 * NNODE, [[1, NNODE], [0, P]])
                    diag_out = _mkap(diag[:], 0, [[HEADS * P, NNODE], [1, P]])
                    nc.gpsimd.affine_select(
                        out=diag_out, in_=attn_in,
                        pattern=[[0, NNODE], [1, P]],
                        compare_op=ALU.is_equal, fill=0.0,
                        base=0, channel_multiplier=-1)
                    attn_in1 = _mkap(attn[:], t * 2 * NNODE + NNODE,
                                     [[1, NNODE], [0, P]])
                    diag_out1 = _mkap(diag[:], P, [[HEADS * P, NNODE], [1, P]])
                    nc.gpsimd.affine_select(
                        out=diag_out1, in_=attn_in1,
                        pattern=[[0, NNODE], [1, P]],
                        compare_op=ALU.is_equal, fill=0.0,
                        base=0, channel_multiplier=-1)
                    pair, half = (t // 2) % 2, t % 2
                    cbase = 1024 + pair * 512 + half * HEADS * P
                    cps = psA[:, cbase:cbase + HEADS * P]
                    for c in range(NNODE):
                        k = t * NNODE + c
                        nc.tensor.matmul(
                            cps,
                            emb[:, k * EW:k * EW + H],
                            diag[:, c * HEADS * P:(c + 1) * HEADS * P],
                            start=(c == 0), stop=(c == NNODE - 1))
                    if half == 1:
                        t0 = t - 1
                        ctx_out = _mkap(ctx[:], t0 * P,
                                        [[P, 2], [bc, HEADS], [1, P]])
                        ps_in = bass.AP(
                            tensor=psA.ap().tensor,
                            offset=1024 + pair * 512,
                            ap=[list(psA.ap().ap[0]),
                                [HEADS * P, 2], [P, HEADS], [1, P]])
                        nc.vector.tensor_copy(ctx_out, ps_in)

            def ph_tail(q):
                tlo, thi = q * QT, min((q + 1) * QT, t_)
                qt = thi - tlo
                # value / policy hidden (bank B3)
                i0, i1 = tlo * P, thi * P
                for (w_sb, bias_sb, out_bf) in ((bv_sb, vb1_sb, vh),
                                                (bp_sb, pb1_sb, ph)):
                    pv = psB[:, 1536:1536 + (i1 - i0)]
                    for h in range(HEADS):
                        nc.tensor.matmul(
                            pv,
                            w_sb[:, h * H:(h + 1) * H],
                            ctx[:, h * bc + i0:h * bc + i1],
                            start=(h == 0), stop=(h == HEADS - 1))
                    nc.scalar.activation(out=out_bf[:, i0:i1], in_=pv,
                                         func=AF.Relu, bias=bias_sb[:],
                                         scale=1.0)
                # output layer [128b, 9] (banks A0/A1)
                ob = (q % 2) * 512
                for s in range(tlo, thi):
                    o0 = ob + (s - tlo) * 9
                    nc.tensor.matmul(psA[:, o0:o0 + 1],
                                     vh[:, s * P:(s + 1) * P],
                                     wv2_sb[:], start=True, stop=True)
                    nc.tensor.matmul(psA[:, o0 + 1:o0 + 9],
                                     ph[:, s * P:(s + 1) * P],
                                     wp2_sb[:], start=True, stop=True)
                outb_b = _mkap(outb_sb[:], 0, [[0, qt], [1, 9]])
                ps_in = bass.AP(tensor=psA.ap().tensor, offset=ob,
                                ap=[list(psA.ap().ap[0]), [9, qt], [1, 9]])
                nc.vector.tensor_tensor(
                    _mkap(outs[:], tlo * 9, [[9, qt], [1, 9]]),
                    ps_in, outb_b, op=ALU.add)
                # outputs to DRAM
                val_src = _mkap(outs[:], tlo * 9, [[9, qt]])
                val_dst = bass.AP(tensor=d_val.tensor, offset=tlo * P,
                                  ap=[[1, P], [P, qt]])
                nc.sync.dma_start(out=val_dst, in_=val_src)
                log_src = _mkap(outs[:], tlo * 9 + 1, [[9, qt], [1, ACTD]])
                log_dst = bass.AP(tensor=d_log.tensor,
                                  offset=tlo * P * ACTD,
                                  ap=[[ACTD, P], [ACTD * P, qt], [1, ACTD]])
                nc.sync.dma_start(out=log_dst, in_=log_src)

            # software-pipelined emission: skew phases by one/two quarters
            for w in range(nq + 2):
                if w < nq:
                    ph_front(w)
                if 1 <= w <= nq:
                    ph_attn(w - 1)
                if w >= 2:
                    ph_tail(w - 2)

    nc.compile()
    nc._dbg_aps = {
        "aobsT": aobsT, "relu1": relu1, "emb": emb, "attn": attn,
        "ctx": ctx, "vh": vh, "ph": ph, "outs": outs, "adjm1": adjm1,
    }
    return nc


def _precompute(enc_w1, enc_b1, enc_w2, enc_b2, gat_w, gat_a,
                val_w1, val_b1, val_w2, val_b2,
                pol_w1, pol_b1, pol_w2, pol_b2):
    bfdt = ml_dtypes.bfloat16
    f = np.float32
    a1, a2 = gat_a[:, :H].astype(f), gat_a[:, H:].astype(f)
    va1 = np.stack([gat_w[h] @ a1[h] for h in range(HEADS)])
    va2 = np.stack([gat_w[h] @ a2[h] for h in range(HEADS)])
    u1 = np.stack([enc_w2 @ va1[h] for h in range(HEADS)])
    u2 = np.stack([enc_w2 @ va2[h] for h in range(HEADS)])
    kb = np.array([[enc_b2 @ va1[h] + enc_b2 @ va2[h]
                    for h in range(HEADS)]], f)
    w2aug = np.concatenate(
        [enc_w2] + [c[:, None] for pair in zip(u1, u2) for c in pair],
        axis=1).astype(f)
    bv = np.concatenate([gat_w[h] @ val_w1[h * H:(h + 1) * H]
                         for h in range(HEADS)], axis=1).astype(f)
    bp = np.concatenate([gat_w[h] @ pol_w1[h * H:(h + 1) * H]
                         for h in range(HEADS)], axis=1).astype(f)
    vb1 = (val_b1 + enc_b2 @ (bv[:, :H] + bv[:, H:])).astype(f)[:, None]
    pb1 = (pol_b1 + enc_b2 @ (bp[:, :H] + bp[:, H:])).astype(f)[:, None]
    outb = np.concatenate([val_b2, pol_b2]).astype(f)[None, :]
    w1p = np.zeros((P, H), f)
    w1p[:OBS] = enc_w1
    wpb = np.concatenate(
        [w1p, np.vstack([w2aug] * 1), bv, bp,
         val_w2, pol_w2, np.eye(P, dtype=f)], axis=1).astype(bfdt)
    wpf = np.concatenate(
        [enc_b1[:, None], vb1, pb1,
         np.tile(kb, (P, 1)), np.tile(outb, (P, 1))], axis=1).astype(f)
    return {"wpb": np.ascontiguousarray(wpb),
            "wpf": np.ascontiguousarray(wpf)}


def kernel(obs, neighbor_obs, adj,
           enc_w1, enc_b1, enc_w2, enc_b2, gat_w, gat_a,
           val_w1, val_b1, val_w2, val_b2,
           pol_w1, pol_b1, pol_w2, pol_b2,
           _trace=False, _trace_kwargs=None):
    from concourse.bass_utils import run_bass_kernel_spmd

    if "nc" not in _CACHE:
        _CACHE["nc"] = _build()
    nc = _CACHE["nc"]

    wmap = _precompute(enc_w1, enc_b1, enc_w2, enc_b2, gat_w, gat_a,
                       val_w1, val_b1, val_w2, val_b2,
                       pol_w1, pol_b1, pol_w2, pol_b2)
    obs = np.asarray(obs, np.float32)
    nbr = np.asarray(neighbor_obs, np.float32)
    adj = np.ascontiguousarray(np.asarray(adj, np.int32))

    # [B,5,64] -> per-core [64, T*5*128], node-major columns
    aobs = np.concatenate([obs[:, None, :], nbr], axis=1)
    aobsT = np.ascontiguousarray(
        aobs.reshape(N_CORES, BC // P, P, NNODE, OBS)
            .transpose(0, 4, 1, 3, 2)
            .reshape(N_CORES, OBS, BC * NNODE)
            .astype(ml_dtypes.bfloat16))

    in_maps = []
    for c in range(N_CORES):
        s = slice(c * BC, (c + 1) * BC)
        in_maps.append({"aobsT": aobsT[c], "adj": adj[s], **wmap})

    kw = {}
    if _trace:
        kw = dict(trace=True, **(_trace_kwargs or {}))
    res = run_bass_kernel_spmd(nc, in_maps, list(range(N_CORES)), **kw)
    value = np.concatenate([r["value"] for r in res.results], axis=0)
    logits = np.concatenate([r["logits"] for r in res.results], axis=0)
    _CACHE["last_results"] = res
    return value, logits
